# revision 1
# baseline (speedup 1.0000x reference)
"""2x bilinear upsample (half_pixel_centers=False) on Trainium2.

Input  x: [16, 64, 128, 128] f32  ->  Output: [16, 64, 256, 256] f32.

With scale=2 and the legacy (no half-pixel offset) coordinate map
h_src = 0.5 * h_dst, the op reduces to:
  out[2i, 2j]     = x[i, j]
  out[2i, 2j+1]   = 0.5*(x[i,j]   + x[i,j+1])     (clamped at right edge)
  out[2i+1, k]    = 0.5*(Y[i, k]  + Y[i+1, k])    (clamped at bottom edge)
where Y is the row (horizontally) upsampled image.

Sharding: pure data parallel, batch 16 -> 2 samples per core x 8 cores.

Per-core layout: the 128 images (2 samples x 64 channels) sit on the 128
SBUF partitions; H x W flattened along the free dimension.  All neighbor
averaging is then free-dim-only (no cross-partition movement) and every
DMA is contiguous runs >= 1KB per partition.

Bit-exactness: we build T = 0.5*Y using only power-of-2 scalings (exact
in fp32):  T_even_col = 2*(0.25*x),  T_odd_col = 0.25*x_j + 0.25*x_{j+1}
(= 0.5 * fl(0.5 x_j + 0.5 x_{j+1}) exactly, by binary-scaling invariance
of round-to-nearest).  Then even output rows = 2*T (exact) and odd rows
= T_r + T_{r+1} = fl(0.5 Y_r + 0.5 Y_{r+1}), matching the reference's
rounding exactly.

Engine/tile flow (Bacc's generate_event_semaphores legalizes the sync
waits for the TRN2 ISA's 1-wait-per-instruction limit):
  I  : written by SP-ring DMA (loads), read only by ACT (Iq = 0.25*I)
  Iq : written by ACT, read only by DVE
  T  : written by DVE (x3), read by ACT (even out rows) + DVE (odd)
  O  : even rows by ACT, odd rows by DVE, stored on the ACT HWDGE ring
       as one contiguous per-partition run per slab

Each slab loads exactly its RS input rows (no overlap): the odd output
row straddling a slab boundary is computed by the next slab from
T_prev[RS-1] + T[0] (same-engine DVE ordering makes this dependency
free), and each store window shifts down one row to stay contiguous.
"""

import numpy as np

from concourse import bacc, mybir
from concourse import bass_utils
from concourse.tile import TileContext

N, C, H, W = 16, 64, 128, 128
OH, OW = 2 * H, 2 * W
NCORES = 8
NS = N // NCORES          # samples per core
P = NS * C                # 128 images per core = partition count
RS = 8                    # input rows per slab
NSLAB = H // RS           # 8 slabs

_f32 = mybir.dt.float32
_nc_cache = {}


def _build():
    nc = bacc.Bacc("TRN2", target_bir_lowering=False)
    x = nc.dram_tensor("x", (NS, C, H, W), _f32, kind="ExternalInput")
    out = nc.dram_tensor("out", (NS, C, OH, OW), _f32, kind="ExternalOutput")

    xr = x[:].rearrange("n c h w -> (n c) h w")      # [128, 128, 128]
    outr = out[:].rearrange("n c h w -> (n c) h w")  # [128, 256, 256]

    bi, bq, bt, bo = (4, 2, 2, 2) if RS >= 16 else (6, 3, 3, 4)
    with TileContext(nc) as tc:
        with tc.tile_pool(name="pin", bufs=bi) as pin, \
             tc.tile_pool(name="piq", bufs=bq) as piq, \
             tc.tile_pool(name="pt", bufs=bt) as pt, \
             tc.tile_pool(name="po", bufs=bo) as po:
            t3_prev = None
            for s in range(NSLAB):
                first = s == 0
                last = s == NSLAB - 1
                # slab s emits output rows [out0, out0 + rows_out):
                #   boundary odd row 2*RS*s-1 (s>0), its RS even rows,
                #   its RS-1 interior odd rows, and row OH-1 (last slab)
                out0 = 0 if first else 2 * RS * s - 1
                eoff = 0 if first else 1   # even rows start here in o3
                rows_out = eoff + 2 * RS - 1 + (1 if last else 0)

                ti = pin.tile([P, RS * W], _f32, tag="i")
                tq = piq.tile([P, RS * W], _f32, tag="q")
                tt = pt.tile([P, RS * OW], _f32, tag="t")
                to = po.tile([P, rows_out * OW], _f32, tag="o")

                i3 = ti[:].rearrange("p (r w) -> p r w", w=W)
                q3 = tq[:].rearrange("p (r w) -> p r w", w=W)
                t3 = tt[:].rearrange("p (r w) -> p r w", w=OW)
                o3 = to[:].rearrange("p (r w) -> p r w", w=OW)

                # load input rows [RS*s, RS*(s+1))   (SP HWDGE ring)
                nc.sync.dma_start(i3, xr[:, RS * s:RS * (s + 1), :])

                # Iq = 0.25 * I   (ACT; sole reader of I)
                nc.scalar.mul(tq[:], ti[:], 0.25)

                # T even cols = 2*Iq = 0.5*I   (DVE)
                nc.vector.tensor_scalar_mul(t3[:, :, 0:OW:2], q3, 2.0)
                # T odd cols j<127: Iq_j + Iq_{j+1}   (DVE)
                nc.vector.tensor_add(
                    t3[:, :, 1:OW - 1:2], q3[:, :, 0:W - 1], q3[:, :, 1:W])
                # T last col = 2*Iq last col   (DVE, tiny)
                nc.vector.tensor_scalar_mul(
                    t3[:, :, OW - 1:OW], q3[:, :, W - 1:W], 2.0)

                # boundary odd row (first row of this store window, s>0):
                # T_prev[RS-1] + T[0]
                if not first:
                    nc.vector.tensor_add(
                        o3[:, 0:1, :], t3_prev[:, RS - 1:RS, :], t3[:, 0:1, :])
                # even output rows = 2 * T_r   (ACT)
                nc.scalar.mul(
                    o3[:, eoff:eoff + 2 * RS - 1:2, :], t3[:, 0:RS, :], 2.0)
                # interior odd rows = T_r + T_{r+1}   (DVE)
                nc.vector.tensor_add(
                    o3[:, eoff + 1:eoff + 2 * RS - 2:2, :],
                    t3[:, 0:RS - 1, :], t3[:, 1:RS, :])
                if last:
                    # bottom edge: out row OH-1 = Y[H-1] = 2*T[RS-1]
                    nc.scalar.mul(
                        o3[:, rows_out - 1:rows_out, :],
                        t3[:, RS - 1:RS, :], 2.0)

                # store rows [out0, out0 + rows_out): one contiguous run
                # per partition (ACT HWDGE ring)
                nc.scalar.dma_start(
                    outr[:, out0:out0 + rows_out, :], to[:])
                t3_prev = t3
    nc.compile()
    return nc


def kernel(x: np.ndarray, _trace=False, _trace_kwargs=None):
    if "nc" not in _nc_cache:
        _nc_cache["nc"] = _build()
    nc = _nc_cache["nc"]

    x = np.ascontiguousarray(np.asarray(x, dtype=np.float32))
    in_maps = [{"x": x[NS * i:NS * (i + 1)]} for i in range(NCORES)]
    res = bass_utils.run_bass_kernel_spmd(
        nc, in_maps, core_ids=list(range(NCORES)), trace=_trace,
        **(_trace_kwargs or {}))
    out = np.concatenate([r["out"] for r in res.results], axis=0)
    if _trace:
        return out, res
    return out

